# revision 4
# baseline (speedup 1.0000x reference)
"""Trainium2 Bass kernel: single-head causal self-attention.

Problem: x:(8,2048,1024) f32, Wk/Wq/Wv:(1024,64) f32
  k,q,v = x@Wk, x@Wq, x@Wv ; S = q k^T / sqrt(64) causal-masked
  out = softmax(S) @ v  -> (8,2048,64) f32

Sharding: data-parallel over batch B=8 across the 8 NeuronCores (one batch
element per core).

Per-core design (fp16 matmul paths, fp32 PSUM accumulation):
  - Host pre-tiles x^T into chunk-major (NBLK, P, CT*QB) so each 512-query
    chunk is one fully-contiguous 1MB DMA (split into two 64-partition
    halves across queues for bandwidth). Weights land in one contiguous
    (P, CT*192) transfer. DMA issue cost is ~650ns/instruction and DRAM
    bursts need large contiguous streams, so few+large+contiguous wins.
  - Warm-up matmuls bridge the preamble->chunk-0 window so the PE HAM
    clock-gate never drops to half speed.
  - Projections per chunk: psum(128,512) = [Wk|Wv]_c^T @ x^T_c accumulated
    over 8 c-tiles -> rows 0:64 k^T, 64:128 v^T (packed "kvt"); q^T (M=64)
    separately. Chunk g+1 projections interleave into attention block g-…
    as background thunks.
  - v^T -> v natural via PE transpose against an identity block; a
    ones-column is appended (V') so the PV matmul also emits the softmax
    denominator as row 64.
  - Scores transposed: S^T_j = K_j Q^T per 512-query block; causally-dead
    column ranges are never computed. Adjacent key tiles (2j,2j+1) share a
    2-bank PSUM pair so one scalar-engine exp covers both (halves
    activation count); diagonal/dead regions are zeroed after exp by
    widened gpsimd affine_selects.
  - out'^T = V'^T P^T accumulated in PSUM over key tiles; epilogue
    PE-transposes to natural, divides by the denominator column, and DMAs
    each block as a contiguous (P, 4*H) f32 slab of a (NBLK, P, 4*H)
    output; the host de-interleaves to (T, H).
"""

import os
import sys
from contextlib import ExitStack

import numpy as np

if "/opt/trn_rl_repo" not in sys.path:
    sys.path.insert(0, "/opt/trn_rl_repo")

import concourse.bacc as bacc
import concourse.bass as bass
import concourse.mybir as mybir
import concourse.tile as tile
from concourse.bass import ds
from concourse.bass_utils import run_bass_kernel_spmd
from concourse.masks import make_identity

F32 = mybir.dt.float32
F16 = mybir.dt.float16

B, T, C, H = 8, 2048, 1024, 64
P = 128           # partitions
CT = C // P       # 8 c-tiles
NBLK = 4          # query blocks of 512
QB = T // NBLK    # 512 queries per block
KT = T // P       # 16 key tiles
SCALE = H ** -0.5
N_WARM = 8
WCOL = 256        # warm-up matmul width


def build_bass():
    nc = bacc.Bacc("TRN2")

    xtt = nc.dram_tensor("xtt", (NBLK, P, CT * QB), F16, kind="ExternalInput")
    wt = nc.dram_tensor("wt", (P, CT * (2 * H + H)), F16, kind="ExternalInput")
    out = nc.dram_tensor("out", (NBLK, P, 4 * H), F32, kind="ExternalOutput")

    with ExitStack() as ctx:
        tc = ctx.enter_context(tile.TileContext(nc))
        const = ctx.enter_context(tc.tile_pool(name="const", bufs=1))
        ptp = ctx.enter_context(tc.tile_pool(name="ptp", bufs=3))
        sml = ctx.enter_context(tc.tile_pool(name="sml", bufs=2))
        psS = ctx.enter_context(tc.tile_pool(name="psS", bufs=2, space="PSUM"))
        psP = ctx.enter_context(tc.tile_pool(name="psP", bufs=2, space="PSUM"))
        psO = ctx.enter_context(tc.tile_pool(name="psO", bufs=1, space="PSUM"))
        psT = ctx.enter_context(tc.tile_pool(name="psT", bufs=1, space="PSUM"))

        # ---- persistent SBUF ----
        xt_sb = const.tile([P, NBLK, CT, QB], F16)   # x^T chunk-major
        w_sb = const.tile([P, CT, 3 * H], F16)       # [Wk|Wv|Wq] c-tiles
        kvt = const.tile([P, T], F16)                # rows 0:64 k^T, 64:128 v^T
        qt = const.tile([H, T], F16)                 # q^T
        vsb = const.tile([P, KT, H + 1], F16)        # V' tiles (v | ones-col)
        outn = const.tile([P, KT, H], F32)           # natural out tiles
        ident = const.tile([P, P], F16)
        wrm = const.tile([P, WCOL], F16)             # warm-up operand

        # ---- constants (no DMA deps -> issue immediately) ----
        nc.gpsimd.memset(wrm[:], 0.25)
        make_identity(nc, ident)
        nc.gpsimd.memset(vsb[:, :, H:H + 1], 1.0)    # V' ones-column

        # ---- input DMA: few large contiguous transfers ----
        # chunk g = two 64-partition halves (each a contiguous 512KB stream)
        xv = xt_sb.rearrange("p n c q -> p (n c q)")
        def xdma(eng, g, half):
            rows = ds(half * 64, 64)
            eng.dma_start(
                xt_sb[rows, g, :, :], xtt[g, rows, :])
        nc.gpsimd.dma_start(w_sb[:], wt.rearrange("p (c m) -> p c m", m=3 * H))
        xdma(nc.scalar, 0, 0)
        xdma(nc.sync, 0, 1)
        xdma(nc.gpsimd, 1, 0)
        xdma(nc.gpsimd, 1, 1)
        xdma(nc.scalar, 2, 0)
        xdma(nc.sync, 2, 1)
        xdma(nc.scalar, 3, 0)
        xdma(nc.sync, 3, 1)

        # ---- PE warm-up while chunk 0 loads: keeps the HAM clock alive ----
        for _ in range(N_WARM):
            pw = psT.tile([P, WCOL], F32, tag="tr")
            nc.tensor.matmul(pw[:], wrm[:, 0:P], wrm[:], start=True, stop=True)

        def proj_chunk(g):
            sl = ds(g * QB, QB)
            pk = psP.tile([P, QB], F32, tag="mm")
            for c in range(CT):
                nc.tensor.matmul(pk[:], w_sb[:, c, 0:2 * H], xt_sb[:, g, c, :],
                                 start=(c == 0), stop=(c == CT - 1))
            nc.vector.tensor_copy(kvt[:, sl], pk[:])
            pq = psP.tile([H, QB], F32, tag="mm")
            for c in range(CT):
                nc.tensor.matmul(pq[:], w_sb[:, c, 2 * H:3 * H], xt_sb[:, g, c, :],
                                 start=(c == 0), stop=(c == CT - 1))
            nc.vector.tensor_copy(qt[:, sl], pq[:])

        def v_nat(g):
            # 4 transposed v chunks into one psum, single batched copy out
            pn = psT.tile([P, 4, H], F32, tag="tr")
            for i in range(4):
                t = 4 * g + i
                nc.tensor.matmul(pn[:, i, :], kvt[H:P, ds(t * P, P)],
                                 ident[H:P, H:H + H], start=True, stop=True)
            nc.vector.tensor_copy(vsb[:, ds(4 * g, 4), 0:H], pn[:])

        def make_bg(g):
            # thunks that project chunk g / build its V' tiles; interleaved
            # into the previous attention block so projections overlap the
            # scalar-engine exp work
            sl = ds(g * QB, QB)
            pk = psP.tile([P, QB], F32, tag="mm")
            pq = psP.tile([H, QB], F32, tag="mm")
            th = []
            for c in range(CT):
                th.append(lambda c=c: nc.tensor.matmul(
                    pk[:], w_sb[:, c, 0:2 * H], xt_sb[:, g, c, :],
                    start=(c == 0), stop=(c == CT - 1)))
            th.append(lambda: nc.vector.tensor_copy(kvt[:, sl], pk[:]))
            for c in range(CT):
                th.append(lambda c=c: nc.tensor.matmul(
                    pq[:], w_sb[:, c, 2 * H:3 * H], xt_sb[:, g, c, :],
                    start=(c == 0), stop=(c == CT - 1)))
            th.append(lambda: nc.vector.tensor_copy(qt[:, sl], pq[:]))
            pn = psT.tile([P, 4, H], F32, tag="tr")
            for i in range(4):
                th.append(lambda i=i: nc.tensor.matmul(
                    pn[:, i, :], kvt[H:P, ds((4 * g + i) * P, P)],
                    ident[H:P, H:H + H], start=True, stop=True))
            th.append(lambda: nc.vector.tensor_copy(vsb[:, ds(4 * g, 4), 0:H],
                                                    pn[:]))
            return th

        def attn_block(b, bg=()):
            po = psO.tile([H + 1, QB], F32, tag="o")
            npair = 2 * b + 2
            prev = None

            def pv(pt, m):
                for i in (0, 1):
                    j = 2 * m + i
                    c0 = max(0, P * j - QB * b)
                    nc.tensor.matmul(po[:, c0:], vsb[:, j, :], pt[:, i, c0:],
                                     start=(m == 0 and i == 0),
                                     stop=(m == npair - 1 and i == 1))

            per = -(-len(bg) // npair)
            for m in range(npair):
                j0, j1 = 2 * m, 2 * m + 1
                c00 = max(0, P * j0 - QB * b)
                c01 = max(0, P * j1 - QB * b)
                ps = psS.tile([P, 2, QB], F32, tag="s")
                nc.tensor.matmul(ps[:, 0, c00:], kvt[0:H, ds(j0 * P, P)],
                                 qt[:, ds(b * QB + c00, QB - c00)],
                                 start=True, stop=True)
                nc.tensor.matmul(ps[:, 1, c01:], kvt[0:H, ds(j1 * P, P)],
                                 qt[:, ds(b * QB + c01, QB - c01)],
                                 start=True, stop=True)
                # one exp over the whole pair; j1's [c00,c01) cols are psum
                # garbage here and get zeroed by the widened affine_select
                pt = ptp.tile([P, 2, QB], F16, tag="pt")
                nc.scalar.activation(pt[:, :, c00:], ps[:, :, c00:],
                                     mybir.ActivationFunctionType.Exp,
                                     scale=SCALE)
                if P * j0 >= QB * b:  # j0 diagonal chunk
                    nc.gpsimd.affine_select(
                        out=pt[:, 0, ds(c00, P)], in_=pt[:, 0, ds(c00, P)],
                        compare_op=mybir.AluOpType.is_ge, fill=0.0,
                        base=0, pattern=[[1, P]], channel_multiplier=-1,
                    )
                if P * j1 >= QB * b:  # j1 dead cols [c00,c01) + diagonal
                    w = min(QB, c01 + P) - c00
                    nc.gpsimd.affine_select(
                        out=pt[:, 1, ds(c00, w)], in_=pt[:, 1, ds(c00, w)],
                        compare_op=mybir.AluOpType.is_ge, fill=0.0,
                        base=c00 - c01, pattern=[[1, w]], channel_multiplier=-1,
                    )
                if prev is not None:
                    pv(*prev)
                prev = (pt, m)
                for th in bg[per * m: per * (m + 1)]:
                    th()
            pv(*prev)

            # epilogue: transpose to natural, divide by denominator column
            posb = sml.tile([H + 1, QB], F16, tag="os")
            nc.vector.tensor_copy(posb[:], po[:])
            pn = psT.tile([P, 4, H + 1], F32, tag="tr")
            for i in range(4):
                nc.tensor.matmul(pn[:, i, :], posb[:, ds(i * P, P)],
                                 ident[0:H + 1, 0:H + 1], start=True, stop=True)
            onat = sml.tile([P, 4, H + 1], F32, tag="on")
            nc.vector.tensor_copy(onat[:], pn[:])
            rc = sml.tile([P, 4, 1], F32, tag="rc")
            nc.vector.reciprocal(rc[:], onat[:, :, H:H + 1])
            nc.vector.tensor_tensor(outn[:, ds(4 * b, 4), :],
                                    onat[:, :, 0:H],
                                    rc[:].to_broadcast((P, 4, H)),
                                    mybir.AluOpType.mult)
            nc.sync.dma_start(out[b], outn[:, ds(4 * b, 4), :])

        proj_chunk(0)
        v_nat(0)
        for b in range(NBLK):
            bg = make_bg(b + 1) if b + 1 < NBLK else []
            attn_block(b, bg)

    nc.compile()
    return nc


_NC = None
LAST_EXEC_TIME_NS = None  # filled when BASS_TRACE=1 (read by test.py)
LAST_RESULT = None


def _get_nc():
    global _NC
    if _NC is None:
        _NC = build_bass()
    return _NC


def kernel(x, Wk, Wq, Wv):
    global LAST_EXEC_TIME_NS, LAST_RESULT
    x = np.ascontiguousarray(x, dtype=np.float16)
    wkv = np.concatenate([Wk, Wv], axis=1).astype(np.float16)
    wq = np.asarray(Wq, dtype=np.float16)
    wh = np.concatenate([wkv.reshape(CT, P, 2 * H),
                         wq.reshape(CT, P, H)], axis=2)
    wh = np.ascontiguousarray(wh.transpose(1, 0, 2).reshape(P, CT * 3 * H))

    in_maps = []
    for b in range(B):
        xtt = (x[b].T.reshape(CT, P, NBLK, QB)
               .transpose(2, 1, 0, 3).reshape(NBLK, P, CT * QB))
        in_maps.append({
            "xtt": np.ascontiguousarray(xtt),
            "wt": wh,
        })

    nc = _get_nc()
    res = run_bass_kernel_spmd(nc, in_maps, list(range(B)))
    LAST_EXEC_TIME_NS = res.exec_time_ns
    LAST_RESULT = res
    # out is (NBLK, P, 4*H) block-major; de-interleave to (T, H)
    o = np.stack([np.ascontiguousarray(m["out"]) for m in res.results])
    o = o.reshape(B, NBLK, P, 4, H).transpose(0, 1, 3, 2, 4).reshape(B, T, H)
    return o.astype(np.float32)


# revision 17
# speedup vs baseline: 1.1068x; 1.1068x over previous
"""Trainium2 Bass kernel: single-head causal self-attention.

Problem: x:(8,2048,1024) f32, Wk/Wq/Wv:(1024,64) f32
  k,q,v = x@Wk, x@Wq, x@Wv ; S = q k^T / sqrt(64) causal-masked
  out = softmax(S) @ v  -> (8,2048,64) f32

Sharding: data-parallel over batch B=8 across the 8 NeuronCores (one batch
element per core).

Per-core design:
  - Host pre-tiles x^T chunk+c-tile-major (NBLK, CT, P, QB) so every DMA
    piece is a fully contiguous DRAM stream. Chunk 0 streams as 8 c-tile
    pieces round-robin over the 3 DMA queues (scalar/sync/gpsimd HWDGE) so
    the projection's c-loop can chase arrivals; chunks 1-3 stream as
    c-group thirds. Weights are one contiguous (P, CT*192) transfer.
  - Warm-up matmuls bridge the ~6.5us NEFF preamble -> chunk-0 window so
    the PE HAM clock-gate never drops to half speed.
  - Projections per chunk: psum(128,512) = [Wk|Wv]_c^T @ x^T_c accumulated
    over 8 c-tiles -> rows 0:64 k^T, 64:128 v^T; q^T (M=64) separately.
    Chunk g+1 projections interleave into attention block g as background
    thunks (placed between score and PV work to fill exp-wait bubbles).
  - k^T/q^T are also written as fp8e4 copies with a zeroed second k-tile
    half; score matmuls S^T_j = K_j Q^T then run in DoubleRow perf mode
    (fp8, 2 k-tiles/cycle) for 2x PE throughput on the score phase.
  - v^T -> v natural via PE transpose against an identity block; a
    ones-column is appended (V') so the PV matmul also emits the softmax
    denominator as row 64. PV stays fp16: fp8 v would break the absmax
    error budget for sharply-peaked early rows.
  - Adjacent key tiles (2j,2j+1) share a 2-bank PSUM pair so one
    scalar-engine exp covers both (halves activation count); diagonal/dead
    regions are zeroed after exp by widened gpsimd affine_selects.
  - out'^T = V'^T P^T accumulated in PSUM over key tiles; epilogue
    PE-transposes to natural, divides by the denominator column, and DMAs
    each block as a contiguous (P, 4*H) f32 slab of a (NBLK, P, 4*H)
    output; the host de-interleaves to (T, H).
"""

import os
import sys
from contextlib import ExitStack

import numpy as np

if "/opt/trn_rl_repo" not in sys.path:
    sys.path.insert(0, "/opt/trn_rl_repo")

import concourse.bacc as bacc
import concourse.bass as bass
import concourse.mybir as mybir
import concourse.tile as tile
from concourse.bass import ds
from concourse.bass_utils import run_bass_kernel_spmd
from concourse.masks import make_identity

F32 = mybir.dt.float32
F16 = mybir.dt.float16
F8 = mybir.dt.float8e4
DR = mybir.MatmulPerfMode.DoubleRow

B, T, C, H = 8, 2048, 1024, 64
P = 128           # partitions
CT = C // P       # 8 c-tiles
NBLK = 4          # query blocks of 512
QB = T // NBLK    # 512 queries per block
KT = T // P       # 16 key tiles
SCALE = H ** -0.5
N_WARM = 8
WCOL = 256        # warm-up matmul width

FP8_SCORES = False  # score matmuls via fp8e4 DoubleRow (zero-padded 2nd tile)


def build_bass():
    nc = bacc.Bacc("TRN2")

    xtt = nc.dram_tensor("xtt", (NBLK, P, CT, QB), F16, kind="ExternalInput")
    wkvt = nc.dram_tensor("wkvt", (P, CT * 2 * H), F16, kind="ExternalInput")
    wqt = nc.dram_tensor("wqt", (P, CT * H), F16, kind="ExternalInput")
    out = nc.dram_tensor("out", (NBLK, P, 4 * H), F32, kind="ExternalOutput")

    with ExitStack() as ctx:
        tc = ctx.enter_context(tile.TileContext(nc))
        const = ctx.enter_context(tc.tile_pool(name="const", bufs=1))
        ptp = ctx.enter_context(tc.tile_pool(name="ptp", bufs=3))
        sml = ctx.enter_context(tc.tile_pool(name="sml", bufs=2))
        psS = ctx.enter_context(tc.tile_pool(name="psS", bufs=2, space="PSUM"))
        psP = ctx.enter_context(tc.tile_pool(name="psP", bufs=2, space="PSUM"))
        psO = ctx.enter_context(tc.tile_pool(name="psO", bufs=1, space="PSUM"))
        psT = ctx.enter_context(tc.tile_pool(name="psT", bufs=1, space="PSUM"))

        # ---- persistent SBUF ----
        xt_sb = const.tile([P, NBLK, CT, QB], F16)   # x^T chunk-major
        wkv_sb = const.tile([P, CT, 2 * H], F16)     # [Wk|Wv] c-tiles
        wq_sb = const.tile([P, CT, H], F16)          # Wq c-tiles
        kvt = const.tile([P, T], F16)                # rows 0:64 k^T, 64:128 v^T
        vsb = const.tile([P, KT, H + 1], F16)        # V' tiles (v | ones-col)
        outn = const.tile([P, KT, H], F32)           # natural out tiles
        ident = const.tile([P, P], F16)
        wrm = const.tile([P, WCOL], F16)             # warm-up operand
        if FP8_SCORES:
            k8 = const.tile([H, 2, T], F8)           # [k^T | zeros] fp8
            q8 = const.tile([H, 2, T], F8)           # [q^T | zeros] fp8
        else:
            qt = const.tile([H, T], F16)             # q^T

        # ---- constants (no DMA deps -> issue immediately) ----
        nc.gpsimd.memset(wrm[:], 0.25)
        make_identity(nc, ident)
        nc.gpsimd.memset(vsb[:, :, H:H + 1], 1.0)    # V' ones-column
        if FP8_SCORES:
            nc.gpsimd.memset(k8[:, 1, :], 0.0)       # zero second k-tile
            nc.gpsimd.memset(q8[:, 1, :], 0.0)

        # ---- input DMA: contiguous pieces, chunk 0 first, c-ordered ----
        # scalar+sync HWDGE rings open ~8.5us (post-preamble); gpsimd's
        # software DGE opens ~14.5us, so it only carries the last chunk.
        def xdma(eng, g, c0, c1):
            eng.dma_start(xt_sb[:, g, c0:c1, :], xtt[g, :, c0:c1, :])
        nc.scalar.dma_start(wkv_sb[:],
                            wkvt.rearrange("p (c m) -> p c m", m=2 * H))
        nc.sync.dma_start(wq_sb[:],
                          wqt.rearrange("p (c m) -> p c m", m=H))
        xdma(nc.sync, 0, 4, 6)
        xdma(nc.scalar, 0, 0, 2)
        xdma(nc.sync, 0, 6, 8)
        xdma(nc.scalar, 0, 2, 4)
        xdma(nc.scalar, 1, 0, 4)
        xdma(nc.sync, 1, 4, 8)
        xdma(nc.scalar, 2, 0, 4)
        xdma(nc.sync, 2, 4, 8)
        xdma(nc.gpsimd, 3, 0, 8)
        CORDER = {0: [4, 5, 0, 1, 6, 7, 2, 3],
                  1: [0, 4, 1, 5, 2, 6, 3, 7],
                  2: [0, 4, 1, 5, 2, 6, 3, 7],
                  3: list(range(CT))}

        # ---- PE warm-up while chunk 0 loads: keeps the HAM clock alive ----
        for _ in range(N_WARM):
            pw = psT.tile([P, WCOL], F32, tag="tr")
            nc.tensor.matmul(pw[:], wrm[:, 0:P], wrm[:], start=True, stop=True)

        def proj_thunks(g):
            # kv/q projections + fp8 copies + natural-v for chunk g;
            # the c-loop follows the DMA arrival order for chunk g
            sl = ds(g * QB, QB)
            corder = CORDER[g]
            pk = psP.tile([P, QB], F32, tag="mm")
            pq = psP.tile([H, QB], F32, tag="mm")
            th = []
            for ci, c in enumerate(corder):
                th.append(lambda c=c, ci=ci: nc.tensor.matmul(
                    pk[:], wkv_sb[:, c, :], xt_sb[:, g, c, :],
                    start=(ci == 0), stop=(ci == CT - 1)))
            th.append(lambda: nc.vector.tensor_copy(kvt[:, sl], pk[:]))
            if FP8_SCORES:
                th.append(lambda: nc.vector.tensor_copy(k8[:, 0, sl], pk[0:H, :]))
            for ci, c in enumerate(corder):
                th.append(lambda c=c, ci=ci: nc.tensor.matmul(
                    pq[:], wq_sb[:, c, :], xt_sb[:, g, c, :],
                    start=(ci == 0), stop=(ci == CT - 1)))
            if FP8_SCORES:
                th.append(lambda: nc.vector.tensor_copy(q8[:, 0, sl], pq[:]))
            else:
                th.append(lambda: nc.vector.tensor_copy(qt[:, sl], pq[:]))
            pn = psT.tile([P, 4, H], F32, tag="tr")
            for i in range(4):
                th.append(lambda i=i: nc.tensor.matmul(
                    pn[:, i, :], kvt[H:P, ds((4 * g + i) * P, P)],
                    ident[H:P, H:H + H], start=True, stop=True))
            th.append(lambda: nc.vector.tensor_copy(vsb[:, ds(4 * g, 4), 0:H],
                                                    pn[:]))
            return th

        def score_mm(ps_half, j, b, c0):
            qsl = ds(b * QB + c0, QB - c0)
            if FP8_SCORES:
                nc.tensor.matmul(ps_half, k8[:, :, ds(j * P, P)], q8[:, :, qsl],
                                 start=True, stop=True, perf_mode=DR)
            else:
                nc.tensor.matmul(ps_half, kvt[0:H, ds(j * P, P)], qt[:, qsl],
                                 start=True, stop=True)

        def attn_block(b, bg=()):
            po = psO.tile([H + 1, QB], F32, tag="o")
            npair = 2 * b + 2
            prev = None

            def pv(pt, m):
                for i in (0, 1):
                    j = 2 * m + i
                    c0 = max(0, P * j - QB * b)
                    nc.tensor.matmul(po[:, c0:], vsb[:, j, :], pt[:, i, c0:],
                                     start=(m == 0 and i == 0),
                                     stop=(m == npair - 1 and i == 1))

            per = -(-len(bg) // npair)
            for m in range(npair):
                j0, j1 = 2 * m, 2 * m + 1
                c00 = max(0, P * j0 - QB * b)
                c01 = max(0, P * j1 - QB * b)
                ps = psS.tile([P, 2, QB], F32, tag="s")
                score_mm(ps[:, 0, c00:], j0, b, c00)
                score_mm(ps[:, 1, c01:], j1, b, c01)
                # one exp over the whole pair; j1's [c00,c01) cols are psum
                # garbage here and get zeroed by the widened affine_select
                pt = ptp.tile([P, 2, QB], F16, tag="pt")
                nc.scalar.activation(pt[:, :, c00:], ps[:, :, c00:],
                                     mybir.ActivationFunctionType.Exp,
                                     scale=SCALE)
                if P * j0 >= QB * b:  # j0 diagonal chunk
                    nc.gpsimd.affine_select(
                        out=pt[:, 0, ds(c00, P)], in_=pt[:, 0, ds(c00, P)],
                        compare_op=mybir.AluOpType.is_ge, fill=0.0,
                        base=0, pattern=[[1, P]], channel_multiplier=-1,
                    )
                if P * j1 >= QB * b:  # j1 dead cols [c00,c01) + diagonal
                    w = min(QB, c01 + P) - c00
                    nc.gpsimd.affine_select(
                        out=pt[:, 1, ds(c00, w)], in_=pt[:, 1, ds(c00, w)],
                        compare_op=mybir.AluOpType.is_ge, fill=0.0,
                        base=c00 - c01, pattern=[[1, w]], channel_multiplier=-1,
                    )
                # bg projection work fills the exp-wait bubble before pv(prev)
                for th in bg[per * m: per * (m + 1)]:
                    th()
                if prev is not None:
                    pv(*prev)
                prev = (pt, m)
            pv(*prev)

            # epilogue: transpose to natural, divide by denominator column
            posb = sml.tile([H + 1, QB], F16, tag="os")
            nc.vector.tensor_copy(posb[:], po[:])
            pn = psT.tile([P, 4, H + 1], F32, tag="tr")
            for i in range(4):
                nc.tensor.matmul(pn[:, i, :], posb[:, ds(i * P, P)],
                                 ident[0:H + 1, 0:H + 1], start=True, stop=True)
            onat = sml.tile([P, 4, H + 1], F32, tag="on")
            nc.vector.tensor_copy(onat[:], pn[:])
            rc = sml.tile([P, 4, 1], F32, tag="rc")
            nc.vector.reciprocal(rc[:], onat[:, :, H:H + 1])
            nc.vector.tensor_tensor(outn[:, ds(4 * b, 4), :],
                                    onat[:, :, 0:H],
                                    rc[:].to_broadcast((P, 4, H)),
                                    mybir.AluOpType.mult)
            nc.sync.dma_start(out[b], outn[:, ds(4 * b, 4), :])

        for th in proj_thunks(0):
            th()
        for b in range(NBLK):
            bg = proj_thunks(b + 1) if b + 1 < NBLK else []
            attn_block(b, bg)

    nc.compile()
    return nc


_NC = None
LAST_EXEC_TIME_NS = None  # filled when BASS_TRACE=1 (read by test.py)
LAST_RESULT = None


def _get_nc():
    global _NC
    if _NC is None:
        _NC = build_bass()
    return _NC


def kernel(x, Wk, Wq, Wv):
    global LAST_EXEC_TIME_NS, LAST_RESULT
    x = np.ascontiguousarray(x, dtype=np.float16)
    wkv = np.concatenate([Wk, Wv], axis=1).astype(np.float16)
    wq = np.asarray(Wq, dtype=np.float16)
    wh_kv = np.ascontiguousarray(
        wkv.reshape(CT, P, 2 * H).transpose(1, 0, 2).reshape(P, CT * 2 * H))
    wh_q = np.ascontiguousarray(
        wq.reshape(CT, P, H).transpose(1, 0, 2).reshape(P, CT * H))

    in_maps = []
    for b in range(B):
        xtt = (x[b].T.reshape(CT, P, NBLK, QB)
               .transpose(2, 1, 0, 3))
        in_maps.append({
            "xtt": np.ascontiguousarray(xtt),
            "wkvt": wh_kv,
            "wqt": wh_q,
        })

    nc = _get_nc()
    res = run_bass_kernel_spmd(nc, in_maps, list(range(B)))
    LAST_EXEC_TIME_NS = res.exec_time_ns
    LAST_RESULT = res
    # out is (NBLK, P, 4*H) block-major; de-interleave to (T, H)
    o = np.stack([np.ascontiguousarray(m["out"]) for m in res.results])
    o = o.reshape(B, NBLK, P, 4, H).transpose(0, 1, 3, 2, 4).reshape(B, T, H)
    return o.astype(np.float32)
